# revision 6
# baseline (speedup 1.0000x reference)
"""Trainium2 Bass kernel for out = x @ W.T + b  (x:[8192,1024] f32, W:[1024,1024] f32, b:[1024] f32).

Data-parallel over batch across 8 NeuronCores: each core computes a
[1024,1024] @ [1024,1024]^T matmul + bias for its 1024-row batch shard.

Orientation: W tiles are the stationary operand ([128k x 128o]), x is the
moving operand ([128k x 512b]), so PSUM groups are [128o x 512b] and the
bias is a per-partition scalar (tensor_scalar_add, 4KB bias DMA instead of
a host-replicated 512KB tile).  The output is stored transposed
(out.T [1024o x 1024b] in DRAM) and un-transposed on the host.

Schedule (from perfetto analysis of the previous kernel):
  - PE warm-up matmuls bridge the input-DMA latency and open the HAM
    clock gate; the first real matmul needs only x[k0] (128KB) + the
    leading w[·,k0] slices, so the stream starts ~9.3us instead of 15.4.
  - k-major wavefront: step k runs all 8 o-groups against the single
    x[k] moving tile; per-step feed is a flat 128KB x (sync/Q1 ring) +
    256KB w (scalar/Q10 ring), well under the rings' capacity, so the
    128-matmul stream pipelines at the fp16 floor (~216ns each).
  - The last two k-steps are emitted per-o (staggered closings) so the
    16 bias-adds (alternating vector/gpsimd) and stores (scalar for the
    b0 half, sync for b1) overlap the stream instead of queuing at the
    end; the post-stream drain fits inside the ~3.4us HAM grace window,
    keeping the framework's semaphore-teardown cascade at full clock.
"""

import os

import numpy as np

import concourse.bass as bass
import concourse.mybir as mybir
import concourse.tile as tile
from concourse import bacc
from concourse.bass_utils import run_bass_kernel_spmd

N_CORES = 8
B, IN_F, OUT_F = 8192, 1024, 1024
B_SHARD = B // N_CORES          # 1024 batch rows per core
P = 128                         # SBUF partitions
KO = IN_F // P                  # 8 contraction subtiles
NB = 2                          # 2 batch halves of 512 per core
BI = B_SHARD // NB              # 512 (moving free dim / PSUM bank width)
NO = OUT_F // P                 # 8 output-column tiles of 128
K_TAIL = 4                      # trailing k-steps emitted per-o (staggered)

MODE = os.environ.get("BASS_KERNEL_MODE", "f16")
N_WARMUP = int(os.environ.get("BASS_WARMUP_MMS", "56"))
N_WARMDOWN = int(os.environ.get("BASS_WARMDOWN_MMS", "70"))

_nc_cache = {}


def _build(mode):
    f32 = mybir.dt.float32
    f16 = mybir.dt.float16

    nc = bacc.Bacc("TRN2", target_bir_lowering=False)

    # DRAM layouts are host-packed so every DMA is contiguous per partition:
    #   xt[ki, bh, ko, bi] = x_shard[bh*512 + bi, ko*128 + ki]
    #   wt[ki, ko, ot, oi] = W[ot*128 + oi, ko*128 + ki]
    #   biasr[oi, ot]      = b[ot*128 + oi]
    #   out[o, b]          = result.T  (host un-transposes)
    xt_d = nc.dram_tensor("xt", [P, NB, KO, BI], f16, kind="ExternalInput")
    wt_d = nc.dram_tensor("wt", [P, KO, NO, P], f16, kind="ExternalInput")
    biasr_d = nc.dram_tensor("biasr", [P, NO], f32, kind="ExternalInput")
    out_d = nc.dram_tensor("out", [OUT_F, B_SHARD], f16, kind="ExternalOutput")

    with tile.TileContext(nc) as tc:
        with (
            tc.tile_pool(name="singles", bufs=1) as singles,
            tc.tile_pool(name="wpool", bufs=1) as wpool,
            tc.tile_pool(name="xpool", bufs=1) as xpool,
            tc.tile_pool(name="opool", bufs=NB * NO) as opool,
            tc.tile_pool(name="pspool", bufs=8, space="PSUM") as pspool,
        ):
            scr = singles.tile([P, P], f16)
            nc.vector.memset(scr[:], 0.0)
            bias_sb = singles.tile([P, NO], f32)
            wall = wpool.tile([P, KO, NO, P], f16, name="wall", tag="w_sb")
            xall = xpool.tile([P, NB, KO, BI], f16, name="xall", tag="x_sb")
            o_tiles = [
                opool.tile([P, BI], f16, name=f"o_{g}", tag="o_sb")
                for g in range(NB * NO)
            ]

            # --- PE warm-up: bridges input-DMA latency, opens HAM gate ---
            ps_warm = pspool.tile([P, BI], f32, name="ps_warm", tag="ps")
            for _ in range(N_WARMUP):
                nc.tensor.matmul(ps_warm[:, :64], scr[:], scr[:, :64],
                                 start=True, stop=True)

            # --- input DMA program (need-ordered) ---
            # sync/Q1: x stream + the leading w[k0..k1] (Q10 spins up
            # later and feeds slower, so everything the first couple of
            # steps need rides Q1).  scalar/Q10: w[k2..k7] + bias.
            nc.sync.dma_start(out=xall[:, 0, 0], in_=xt_d[:, 0, 0])
            nc.sync.dma_start(out=wall[:, 0, 0:2], in_=wt_d[:, 0, 0:2])
            nc.scalar.dma_start(out=wall[:, 2], in_=wt_d[:, 2])
            nc.sync.dma_start(out=wall[:, 0, 2:8], in_=wt_d[:, 0, 2:8])
            nc.sync.dma_start(out=xall[:, 0, 1], in_=xt_d[:, 0, 1])
            nc.sync.dma_start(out=wall[:, 1], in_=wt_d[:, 1])
            nc.scalar.dma_start(out=wall[:, 3], in_=wt_d[:, 3])
            for k in range(2, KO):
                nc.sync.dma_start(out=xall[:, 0, k], in_=xt_d[:, 0, k])
                if k + 2 < KO:
                    nc.scalar.dma_start(out=wall[:, k + 2], in_=wt_d[:, k + 2])
            nc.scalar.dma_start(out=bias_sb[:], in_=biasr_d[:])
            for k in range(0, KO, 2):
                nc.sync.dma_start(out=xall[:, 1, k:k + 2],
                                  in_=xt_d[:, 1, k:k + 2])

            # --- matmul wavefront ---
            ps = [None] * NO

            def emit_phase(bh, store_engs):
                for k in range(KO - K_TAIL):
                    for o in range(NO):
                        if k == 0:
                            ps[o] = pspool.tile([P, BI], f32,
                                                name=f"ps_{bh}_{o}", tag="ps")
                        nc.tensor.matmul(
                            ps[o][:],
                            wall[:, k, o],
                            xall[:, bh, k],
                            start=(k == 0),
                            stop=False,
                        )
                for o in range(NO):
                    for k in range(KO - K_TAIL, KO):
                        nc.tensor.matmul(
                            ps[o][:],
                            wall[:, k, o],
                            xall[:, bh, k],
                            start=False,
                            stop=(k == KO - 1),
                        )
                    g = bh * NO + o
                    # group closings are K_TAIL*216ns apart, so a single
                    # DVE handles all psum->sbuf bias-adds queue-free
                    nc.vector.tensor_scalar_add(
                        o_tiles[g][:], ps[o][:], bias_sb[:, o:o + 1]
                    )
                    store_engs[o].dma_start(
                        out=out_d[o * P:(o + 1) * P, bh * BI:(bh + 1) * BI],
                        in_=o_tiles[g][:],
                    )

            emit_phase(0, [nc.sync] * NO)
            emit_phase(1, [nc.scalar, nc.sync] * (NO // 2))

            # --- PE warm-down: keeps the HAM clock gate open through the
            # store drain + the framework's semaphore-teardown cascade,
            # which otherwise runs at half clock. ---
            ps_wd = pspool.tile([P, BI], f32, name="ps_wd", tag="ps")
            for _ in range(N_WARMDOWN):
                nc.tensor.matmul(ps_wd[:, :64], scr[:], scr[:, :64],
                                 start=True, stop=True)
    nc.compile()
    return nc


def _get_nc(mode):
    if mode not in _nc_cache:
        _nc_cache[mode] = _build(mode)
    return _nc_cache[mode]


def _pack(x, W, b, mode="f16"):
    """Shard + retile host-side. Returns in_maps for the 8 cores."""
    x = np.asarray(x, dtype=np.float32)
    W = np.asarray(W, dtype=np.float32)
    b = np.asarray(b, dtype=np.float32)

    # [c, bh, bi, ko, ki] -> [c, ki, bh, ko, bi]
    xs = x.reshape(N_CORES, NB, BI, KO, P).transpose(0, 4, 1, 3, 2)
    # [ot, oi, ko, ki] -> [ki, ko, ot, oi]
    ws = W.reshape(NO, P, KO, P).transpose(3, 2, 0, 1)
    biasr = np.ascontiguousarray(b.reshape(NO, P).T)  # [oi, ot]

    xt = np.ascontiguousarray(xs).astype(np.float16)
    wt = np.ascontiguousarray(ws).astype(np.float16)
    return [{"xt": xt[c], "wt": wt, "biasr": biasr} for c in range(N_CORES)]


def _run(in_maps, mode="f16", **kwargs):
    nc = _get_nc(mode)
    return run_bass_kernel_spmd(nc, in_maps, core_ids=list(range(N_CORES)), **kwargs)


def kernel(x, W, b):
    res = _run(_pack(x, W, b, MODE), MODE)
    # each core returns out.T [1024 o, 1024 b]; un-transpose + concat
    out = np.concatenate([r["out"].T for r in res.results], axis=0)
    return np.ascontiguousarray(out.astype(np.float32))
